# revision 1
# baseline (speedup 1.0000x reference)
"""DCP pooling kernel for Trainium2 (8 NeuronCores, data-parallel over batch).

Math: the reference pads x spatially with zeros, takes |min over channels| of
the padded image, then sums all 3x3 sliding windows (stride 1) and finally
sums everything.  Each padded pixel is covered by cnt(h)*cnt(w) windows where
cnt is 3 in the interior and 2 at the first/last row/col (padded zero pixels
contribute nothing).  So the whole computation collapses to

    sum_{b,h,w} |min_c x[b,c,h,w]| * rw(h) * cw(w)

with rw(h) = 2 if h in {0, H-1} else 3 (same for cw).  A pure streaming
reduction: read 192 MiB, emit one scalar -> memory-bound.

Device program per core (2 images of [3,1024,1024] per core):
  for each of 16 row-tiles [128 rows x 3 channels x 1024 cols] (1.5 MB DMA):
    VectorE: channel-min via two tensor_tensor(min);
    ScalarE: absout = |m| with fused accum_out = per-row sum;
    VectorE: edge-column pair |m|[:,0] + |m|[:,W-1], then accumulate row
    sums / edge sums into a [128,6] block (all tiles / first-row-tile /
    last-row-tile variants so the host can apply the 2-vs-3 row weights).
Host: finish the weighted combine in float64 and sum the 8 cores.
"""

import numpy as np

import concourse.bass as bass
import concourse.bacc as bacc
import concourse.mybir as mybir
from concourse.alu_op_type import AluOpType
from concourse.tile import TileContext
from concourse.bass_utils import run_bass_kernel_spmd

B = 16            # full batch
NCORES = 8
BPC = B // NCORES  # batches per core
C = 3
H = W = 1024
P = 128
NT = H // P       # row-tiles per image

_CACHE: dict = {}


def build_nc(bpc: int = BPC, h: int = H, w: int = W,
             load_bufs: int = 6) -> bass.Bass:
    # Bacc (not plain Bass): its finalize() runs generate_event_semaphores,
    # which splits multi-wait instructions to satisfy the TRN2 1-wait-per-
    # instruction constraint.
    nt = h // P
    nc = bacc.Bacc()
    x = nc.declare_dram_parameter("x", [bpc, C, h, w], mybir.dt.float32,
                                  isOutput=False)
    out = nc.declare_dram_parameter("out", [P, 6], mybir.dt.float32,
                                    isOutput=True)

    with TileContext(nc) as tc:
        with (
            tc.tile_pool(name="loads", bufs=load_bufs) as loads,
            tc.tile_pool(name="work", bufs=2) as work,
            tc.tile_pool(name="small", bufs=4) as small,
            tc.tile_pool(name="accp", bufs=1) as accp,
        ):
            # acc columns: 0 = rowsum over all tiles, 1 = edgesum over all
            # tiles, 2/3 = rowsum/edgesum over t==0 tiles only (host reads
            # partition 0 = image row 0), 4/5 = over t==nt-1 tiles only
            # (host reads partition 127 = image row h-1).
            acc = accp.tile([P, 6], mybir.dt.float32)
            nc.vector.memset(acc[:], 0.0)

            # Explicit zero bias for the Abs activation, initialized on the
            # DVE so the ACT instruction's deps stay on one semaphore.
            zbias = accp.tile([P, 1], mybir.dt.float32)
            nc.vector.memset(zbias[:], 0.0)

            for b in range(bpc):
                for t in range(nt):
                    ctile = loads.tile([P, C, w], mybir.dt.float32,
                                       tag="ctile")
                    src = x[b, :, t * P:(t + 1) * P, :].rearrange(
                        "c p w -> p c w")
                    nc.sync.dma_start(out=ctile[:], in_=src)

                    t1 = work.tile([P, w], mybir.dt.float32, tag="t1")
                    nc.vector.tensor_tensor(t1[:], ctile[:, 0, :],
                                            ctile[:, 1, :], AluOpType.min)
                    m2 = work.tile([P, w], mybir.dt.float32, tag="m2")
                    nc.vector.tensor_tensor(m2[:], t1[:], ctile[:, 2, :],
                                            AluOpType.min)

                    # absout = |m2|, rowsum = sum_w |m2|  (one ACT op)
                    absout = work.tile([P, w], mybir.dt.float32, tag="absout")
                    rowsum = small.tile([P, 1], mybir.dt.float32,
                                        tag="rowsum")
                    nc.scalar.activation(absout[:], m2[:],
                                         mybir.ActivationFunctionType.Abs,
                                         bias=zbias[:],
                                         accum_out=rowsum[:])

                    # |m|(col 0) + |m|(col w-1), per row
                    edge = small.tile([P, 1], mybir.dt.float32, tag="edge")
                    nc.vector.tensor_tensor(edge[:], absout[:, 0:1],
                                            absout[:, w - 1:w],
                                            AluOpType.add)

                    nc.vector.tensor_tensor(acc[:, 0:1], acc[:, 0:1],
                                            rowsum[:], AluOpType.add)
                    nc.vector.tensor_tensor(acc[:, 1:2], acc[:, 1:2],
                                            edge[:], AluOpType.add)
                    if t == 0:
                        nc.vector.tensor_tensor(acc[:, 2:3], acc[:, 2:3],
                                                rowsum[:], AluOpType.add)
                        nc.vector.tensor_tensor(acc[:, 3:4], acc[:, 3:4],
                                                edge[:], AluOpType.add)
                    if t == nt - 1:
                        nc.vector.tensor_tensor(acc[:, 4:5], acc[:, 4:5],
                                                rowsum[:], AluOpType.add)
                        nc.vector.tensor_tensor(acc[:, 5:6], acc[:, 5:6],
                                                edge[:], AluOpType.add)

            nc.sync.dma_start(out=out[:], in_=acc[:])

    nc.finalize()
    return nc


def build_nc_raw(bpc: int = BPC, h: int = H, w: int = W,
                 nbuf: int = 8, detect_races: bool = True) -> bass.Bass:
    """Raw-Bass (no Tile) variant: hand-placed semaphores, no Tile epilogue
    barrier.  Engine programs:
      SP  : pipelined 1.5 MB HWDGE loads (nbuf slots) + final store
      DVE : channel mins + per-tile edge-column reduces + final combine
      ACT : |m| with fused per-row sum -> per-tile rowsum column
            (tiles 0..n-2; the last tile's abs+rowsum runs on the DVE so
            the tail has no cross-engine round-trip)
    Per-tile rowsum/edge values land in distinct columns; one final DVE
    combine collapses them.  The last tile's load is split (c0c1 / c2) so
    tail compute overlaps the final transfer.

    HW pitfall encoded here: a tiny DVE op must not read a location
    written by the IMMEDIATELY preceding DVE op (SBUF write-retire latency
    is exposed between back-to-back short ops and the read sees a stale
    value) -- all short-op chains below keep >=1 intervening op.  Large
    streaming ops are safe (their early elements retire long before the
    next instruction issues).
    """
    from contextlib import ExitStack

    nt = h // P
    n = bpc * nt
    assert n >= 3
    f32 = mybir.dt.float32
    # CoreSim's conservative race detector wants explicit waits even for
    # same-engine program-order deps; it is off for sim validation.
    nc = bacc.Bacc(detect_race_conditions=detect_races)
    x = nc.declare_dram_parameter("x", [bpc, C, h, w], f32, isOutput=False)
    out = nc.declare_dram_parameter("out", [P, 6], f32, isOutput=True)
    tiles = [(b, t) for b in range(bpc) for t in range(nt)]

    with ExitStack() as ctx:
        ec = ctx.enter_context
        ctiles = ec(nc.sbuf_tensor("ctiles", [P, nbuf * C * w], f32))
        t1 = ec(nc.sbuf_tensor("t1", [P, w], f32))
        m2 = ec(nc.sbuf_tensor("m2", [P, 2 * w], f32))
        ab = ec(nc.sbuf_tensor("ab", [P, 2 * w], f32))
        rowsums = ec(nc.sbuf_tensor("rowsums", [P, n], f32))
        edges0 = ec(nc.sbuf_tensor("edges0", [P, n], f32))
        edges1 = ec(nc.sbuf_tensor("edges1", [P, n], f32))
        escr = ec(nc.sbuf_tensor("escr", [P, 2], f32))
        acc = ec(nc.sbuf_tensor("acc", [P, 6], f32))
        zbias = ec(nc.sbuf_tensor("zbias", [P, 1], f32))
        acksink = ec(nc.sbuf_tensor("acksink", [P, 1], f32))
        dma_sems = [ec(nc.semaphore(f"dma_s{i}")) for i in range(nbuf)]
        last01 = ec(nc.semaphore("last01"))
        last2 = ec(nc.semaphore("last2"))
        min2_done = ec(nc.semaphore("min2_done"))
        act_done = ec(nc.semaphore("act_done"))
        fin_done = ec(nc.semaphore("fin_done"))
        out_sem = ec(nc.semaphore("out_sem"))
        block = ec(nc.Block(no_gpsimd_drain=True))

        def src_ap(b, t, c0, c1):
            return x[b, c0:c1, t * P:(t + 1) * P, :].rearrange(
                "c p w -> p c w")

        @block.sync
        def _(sync):
            for i, (b, t) in enumerate(tiles):
                if i >= nbuf:
                    # slot free once DVE consumed tile i-nbuf (min2 done);
                    # the old DMA's completion is covered transitively (DVE
                    # waited on its sem before consuming).
                    sync.wait_ge(min2_done, i - nbuf + 1)
                s = i % nbuf
                base = s * C * w
                if i < n - 1:
                    dst = ctiles[:, base:base + C * w].rearrange(
                        "p (c w) -> p c w", c=C)
                    sync.dma_start(out=dst, in_=src_ap(b, t, 0, C)
                                   ).then_inc(dma_sems[s], 16)
                else:
                    # split last load: c0c1 then c2, so tail compute starts
                    # while c2 is still in flight
                    d01 = ctiles[:, base:base + 2 * w].rearrange(
                        "p (c w) -> p c w", c=2)
                    sync.dma_start(out=d01, in_=src_ap(b, t, 0, 2)
                                   ).then_inc(last01, 16)
                    d2 = ctiles[:, base + 2 * w:base + 3 * w]
                    sync.dma_start(out=d2, in_=src_ap(b, t, 2, 3)[:, 0, :]
                                   ).then_inc(last2, 16)
            sync.wait_ge(fin_done, 1)
            sync.dma_start(out=out[:], in_=acc[:]).then_inc(out_sem, 16)
            sync.wait_ge(out_sem, 16)

        @block.vector
        def _(vector):
            vector.memset(zbias[:], 0.0)
            for i in range(n):
                s = i % nbuf
                base = s * C * w
                c0 = ctiles[:, base:base + w]
                c1 = ctiles[:, base + w:base + 2 * w]
                c2 = ctiles[:, base + 2 * w:base + 3 * w]
                ms = i % 2
                m2s = m2[:, ms * w:(ms + 1) * w]
                if i >= 2:
                    # m2 slot reuse: ACT(i-2) must have read it
                    vector.wait_ge(act_done, i - 1)
                if i < n - 1:
                    vector.wait_ge(dma_sems[s], 16 * (i // nbuf + 1))
                    vector.tensor_tensor(t1[:], c0, c1, AluOpType.min)
                else:
                    vector.wait_ge(last01, 16)
                    vector.tensor_tensor(t1[:], c0, c1, AluOpType.min)
                    vector.wait_ge(last2, 16)
                vector.tensor_tensor(m2s, t1[:], c2,
                                     AluOpType.min).then_inc(min2_done, 1)
                if i == n - 1:
                    # last tile's abs+rowsum on the DVE
                    vector.tensor_reduce(rowsums[:, i:i + 1], m2s[:],
                                         mybir.AxisListType.X, AluOpType.add,
                                         apply_absolute_value=True)
                # per-tile edge columns |m2|[:,0] and |m2|[:,w-1]
                # (two single-element reduces: strided 2-element APs misread
                # on hardware)
                vector.tensor_reduce(edges0[:, i:i + 1], m2s[:, 0:1],
                                     mybir.AxisListType.X, AluOpType.add,
                                     apply_absolute_value=True)
                vector.tensor_reduce(edges1[:, i:i + 1], m2s[:, w - 1:w],
                                     mybir.AxisListType.X, AluOpType.add,
                                     apply_absolute_value=True)

            # final combine; rowsums cols 0..n-2 are ACT's (act_done >= n-1),
            # col n-1 was just written by this engine 3 ops ago
            vector.wait_ge(act_done, n - 1)
            vector.tensor_reduce(acc[:, 0:1], rowsums[:, 0:n],
                                 mybir.AxisListType.X, AluOpType.add)
            vector.tensor_reduce(escr[:, 0:1], edges0[:, 0:n],
                                 mybir.AxisListType.X, AluOpType.add)
            vector.tensor_reduce(escr[:, 1:2], edges1[:, 0:n],
                                 mybir.AxisListType.X, AluOpType.add)
            t0_cols = [b * nt for b in range(bpc)]
            tl_cols = [b * nt + nt - 1 for b in range(bpc)]
            chains = [
                (2, [(rowsums, cc) for cc in t0_cols]),
                (4, [(rowsums, cc) for cc in tl_cols]),
                (3, [(edges0, cc) for cc in t0_cols]
                    + [(edges1, cc) for cc in t0_cols]),
                (5, [(edges0, cc) for cc in tl_cols]
                    + [(edges1, cc) for cc in tl_cols]),
            ]
            for dst, terms in chains:
                buf, cc = terms[0]
                vector.tensor_copy(acc[:, dst:dst + 1], buf[:, cc:cc + 1])
            last = vector.tensor_tensor(acc[:, 1:2], escr[:, 0:1],
                                        escr[:, 1:2], AluOpType.add)
            rounds = max(len(t) for _, t in chains) - 1
            for r in range(rounds):
                for dst, terms in chains:
                    if r + 1 < len(terms):
                        buf, cc = terms[r + 1]
                        last = vector.tensor_tensor(
                            acc[:, dst:dst + 1], acc[:, dst:dst + 1],
                            buf[:, cc:cc + 1], AluOpType.add)
            last.then_inc(fin_done, 1)

        @block.scalar
        def _(scalar):
            for i in range(n - 1):
                scalar.wait_ge(min2_done, i + 1)
                ms = i % 2
                scalar.activation(ab[:, ms * w:(ms + 1) * w],
                                  m2[:, ms * w:(ms + 1) * w],
                                  mybir.ActivationFunctionType.Abs,
                                  bias=zbias[:],
                                  accum_out=rowsums[:, i:i + 1])
                # act_done rides on a trailing copy that READS the accum
                # column: walrus splits the activation into ACTIVATE +
                # READ_ACCUMULATOR, and an inc on the activation itself can
                # fire before the accumulator lands in SBUF.
                scalar.copy(acksink[:], rowsums[:, i:i + 1]
                            ).then_inc(act_done, 1)

    nc.finalize()
    return nc


def _finish_host(results) -> np.float32:
    total = 0.0
    for r in results:
        a = np.asarray(r["out"], dtype=np.float64)
        s_all = 3.0 * a[:, 0].sum() - a[:, 1].sum()  # col-weighted total
        srow_top = 3.0 * a[0, 2] - a[0, 3]     # col-weighted sum of row 0
        srow_bot = 3.0 * a[P - 1, 4] - a[P - 1, 5]   # ... of row H-1
        total += 3.0 * s_all - srow_top - srow_bot
    return np.float32(total)


def kernel(**inputs) -> np.ndarray:
    x = np.ascontiguousarray(np.asarray(inputs["x"], dtype=np.float32))
    assert x.shape == (B, C, H, W), x.shape
    win = int(np.asarray(inputs.get("win_size", 3)))
    assert win == 3, f"kernel specialized for win_size=3, got {win}"

    if "nc" not in _CACHE:
        _CACHE["nc"] = build_nc_raw()
    nc = _CACHE["nc"]

    in_maps = [{"x": x[i * BPC:(i + 1) * BPC]} for i in range(NCORES)]
    res = run_bass_kernel_spmd(nc, in_maps, list(range(NCORES)))
    return np.array(_finish_host(res.results), dtype=np.float32)



# revision 5
# speedup vs baseline: 1.1988x; 1.1988x over previous
"""DCP pooling kernel for Trainium2 (8 NeuronCores, data-parallel over batch).

Math: reference pads x spatially, takes |min over channels| of the padded
image, sums all 3x3 sliding windows, then sums everything.  Padded zeros
contribute nothing, so the result collapses to

    sum_{b,h,w} |min_c x[b,c,h,w]| * rw(h) * cw(w)

with rw(h) = 2 if h in {0, H-1} else 3 (same for cw).  Pure streaming
reduction: read 192 MiB, emit one scalar -> memory-bound.

Layout: each 1024x1024 channel plane is viewed per half as [128, 4096]
(partition p holds 4 consecutive rows: image row = 512*h + 4*p + q,
flat col = q*1024 + w).  That makes every DMA descriptor 8-16KB of
contiguous DRAM per partition - measured ~410-420 GB/s per core vs
~344 GB/s for the old interleaved-channel 4KB-descriptor layout.

Device program per core (2 images = 4 half-planes = "chunks"):
  sync  (SP):   HWDGE loads, [128,2048] col-tiles x 3 channels per tile;
                the last half-plane is split into 6 shrinking col-pieces
                so the post-stream compute tail is tiny.  Final [128,44]
                staging DMA to DRAM.
  vector(DVE):  channel-min per tile (two tensor_tensor.min), plus the
                |col-0|/|col-1023| edge-column extractions for the bulk
                chunks (single-element abs reduces into staging cols).
  scalar(ACT):  Abs activation with fused accum_out per row-group slice
                -> per-(p,q)-rowsum staging cols; piece edge columns.
Host: applies the 2-vs-3 row/col weights in float64 from the staging
columns (per-row sums for rows 0/1023 come out of dedicated q-slices).
"""

import numpy as np

import concourse.bass as bass
import concourse.bacc as bacc
import concourse.mybir as mybir
from concourse.alu_op_type import AluOpType
from concourse.bass_utils import run_bass_kernel_spmd
from contextlib import ExitStack

B = 16            # full batch
NCORES = 8
BPC = B // NCORES  # images per core
C = 3
H = W = 1024
P = 128
F = 4096          # flat cols per half-plane: q*1024 + w, q = 0..3
f32 = mybir.dt.float32

# last half-plane (b=1, h=1) piece split (cols)
PIECES = [(0, 1024), (1024, 2048), (2048, 3072),
          (3072, 3584), (3584, 3840), (3840, 4096)]

# staging columns
RS = list(range(11))   # 0..10: ACT rowsum cols (see build)
RSP5 = 11              # DVE rowsum of [3840:4096] of last half-plane
EDGE0 = 12             # 24 bulk edge cols: chunk c -> 12+8c + [q,side]
PEDGE = 36             # piece edges q0c0,q0c1,q1c0,q1c1,q2c0,q2c1,q3c0
Q3C1 = 43              # last piece's col-1023 edge (DVE)
NCOLS = 44

_CACHE: dict = {}


def build_nc() -> bass.Bass:
    nc = bacc.Bacc(detect_race_conditions=False)
    x = nc.declare_dram_parameter("x", [BPC, C, 2, P, F], f32, isOutput=False)
    out = nc.declare_dram_parameter("out", [P, NCOLS], f32, isOutput=True)

    # full tiles: (b, h, col range); chunk = 2*b + h; pieces cover (1,1)
    fulls = [(b, h, c0, c0 + 2048)
             for (b, h) in [(0, 0), (0, 1), (1, 0)] for c0 in (0, 2048)]

    with ExitStack() as ctx:
        ec = ctx.enter_context
        # 3 rotating trio slots for full tiles, [128, 3*2048] each
        slots = ec(nc.sbuf_tensor("slots", [P, 3 * 3 * 2048], f32))
        # pieces trio buffer for the (1,1) half-plane, [128, 3*4096]
        pslot = ec(nc.sbuf_tensor("pslot", [P, 3 * F], f32))
        t1 = ec(nc.sbuf_tensor("t1", [P, F], f32))
        m2a = ec(nc.sbuf_tensor("m2a", [P, F], f32))   # chunks 0, 2
        m2b = ec(nc.sbuf_tensor("m2b", [P, F], f32))   # chunk 1
        m2c = ec(nc.sbuf_tensor("m2c", [P, F], f32))   # chunk 3 (pieces)
        absout = ec(nc.sbuf_tensor("absout", [P, 3072], f32))
        stag = ec(nc.sbuf_tensor("stag", [P, NCOLS], f32))
        zbias = ec(nc.sbuf_tensor("zbias", [P, 1], f32))
        acksink = ec(nc.sbuf_tensor("acksink", [P, 1], f32))

        csem = [ec(nc.semaphore(f"csem{k}")) for k in range(12)]
        min2_done = ec(nc.semaphore("min2_done"))
        act_done = ec(nc.semaphore("act_done"))
        act_fin = ec(nc.semaphore("act_fin"))
        dve_fin = ec(nc.semaphore("dve_fin"))
        osem = ec(nc.semaphore("osem"))
        block = ec(nc.Block(no_gpsimd_drain=True))

        def slot_ap(k, cols):
            base = (k % 3) * 3 * 2048
            return [slots[:, base + c * 2048:base + c * 2048 + cols]
                    for c in range(C)]

        m2_of = [m2a, m2b, m2a]  # per chunk 0..2

        @block.sync
        def _(sync):
            for k, (b, h, c0, c1) in enumerate(fulls):
                if k >= 3:
                    # trio slot k%3 reused from tile k-3; freed by its min2
                    sync.wait_ge(min2_done, k - 2)
                dsts = slot_ap(k, c1 - c0)
                for c in range(C):
                    sync.dma_start(out=dsts[c], in_=x[b, c, h][:, c0:c1]
                                   ).then_inc(csem[k], 16)
            for j, (c0, c1) in enumerate(PIECES):
                for c in range(C):
                    sync.dma_start(out=pslot[:, c * F + c0:c * F + c1],
                                   in_=x[1, c, 1][:, c0:c1]
                                   ).then_inc(csem[6 + j], 16)
            sync.wait_ge(act_fin, 1)
            sync.wait_ge(dve_fin, 1)
            sync.dma_start(out=out[:], in_=stag[:]).then_inc(osem, 16)
            sync.wait_ge(osem, 16)

        @block.vector
        def _(vector):
            # zbias for ACT's Abs: ACT's first activation waits min2_done>=2,
            # which transitively orders it after this DVE memset.
            vector.memset(zbias[:], 0.0)

            def edge(col, m2, fc):
                vector.tensor_reduce(stag[:, col:col + 1], m2[:, fc:fc + 1],
                                     mybir.AxisListType.X, AluOpType.add,
                                     apply_absolute_value=True)

            for k, (b, h, c0, c1) in enumerate(fulls):
                chunk = 2 * b + h
                m2 = m2_of[chunk]
                s0, s1, s2 = slot_ap(k, c1 - c0)
                vector.wait_ge(csem[k], 48)
                if k == 4:
                    # m2a reuse: chunk-0 rowsum activations must have read it
                    vector.wait_ge(act_done, 2)
                vector.tensor_tensor(t1[:, c0:c1], s0, s1, AluOpType.min)
                vector.tensor_tensor(m2[:, c0:c1], t1[:, c0:c1], s2,
                                     AluOpType.min).then_inc(min2_done, 1)
                # 2 q-groups per 2048-col tile -> 4 edge columns
                qbase = 0 if c0 == 0 else 2
                for qq in range(2):
                    q = qbase + qq
                    base = EDGE0 + 8 * chunk + 2 * q
                    edge(base, m2, q * 1024)
                    edge(base + 1, m2, q * 1024 + 1023)

            for j, (c0, c1) in enumerate(PIECES):
                vector.wait_ge(csem[6 + j], 48)
                vector.tensor_tensor(t1[:, c0:c1], pslot[:, c0:c1],
                                     pslot[:, F + c0:F + c1], AluOpType.min)
                vector.tensor_tensor(m2c[:, c0:c1], t1[:, c0:c1],
                                     pslot[:, 2 * F + c0:2 * F + c1],
                                     AluOpType.min).then_inc(min2_done, 1)
            # last piece epilogue: its rowsum + col-1023 edge
            vector.tensor_reduce(stag[:, RSP5:RSP5 + 1], m2c[:, 3840:4096],
                                 mybir.AxisListType.X, AluOpType.add,
                                 apply_absolute_value=True)
            vector.tensor_reduce(stag[:, Q3C1:Q3C1 + 1], m2c[:, 4095:4096],
                                 mybir.AxisListType.X, AluOpType.add,
                                 apply_absolute_value=True).then_inc(dve_fin, 1)

        @block.scalar
        def _(scalar):
            def act(col, m2, c0, c1, wait=None, inc=False):
                if wait is not None:
                    scalar.wait_ge(min2_done, wait)
                r = scalar.activation(absout[:, 0:c1 - c0], m2[:, c0:c1],
                                      mybir.ActivationFunctionType.Abs,
                                      bias=zbias[:],
                                      accum_out=stag[:, col:col + 1])
                if inc:
                    r.then_inc(act_done, 1)

            # bulk rowsums: h0 chunks split q0 | q123 (row 0 isolation),
            # h1 chunks split q012 | q3 (row 1023 isolation)
            act(0, m2a, 0, 1024, wait=2, inc=True)
            act(1, m2a, 1024, 4096, inc=True)
            act(2, m2b, 0, 3072, wait=4)
            act(3, m2b, 3072, 4096)
            act(4, m2a, 0, 1024, wait=6)
            act(5, m2a, 1024, 4096)
            # pieces of (1,1): rowsums + edge columns
            act(6, m2c, 0, 1024, wait=7)
            act(PEDGE + 0, m2c, 0, 1)
            act(PEDGE + 1, m2c, 1023, 1024)
            act(7, m2c, 1024, 2048, wait=8)
            act(PEDGE + 2, m2c, 1024, 1025)
            act(PEDGE + 3, m2c, 2047, 2048)
            act(8, m2c, 2048, 3072, wait=9)
            act(PEDGE + 4, m2c, 2048, 2049)
            act(PEDGE + 5, m2c, 3071, 3072)
            act(9, m2c, 3072, 3584, wait=10)
            act(PEDGE + 6, m2c, 3072, 3073)
            act(10, m2c, 3584, 3840, wait=11)
            scalar.copy(acksink[:], stag[:, 10:11]).then_inc(act_fin, 1)

    nc.finalize()
    return nc


def make_in_maps(x: np.ndarray) -> list:
    x = np.ascontiguousarray(np.asarray(x, dtype=np.float32))
    return [{"x": x[i * BPC:(i + 1) * BPC].reshape(BPC, C, 2, P, F)}
            for i in range(NCORES)]


def _finish_host(results) -> np.float32:
    total = 0.0
    for r in results:
        a = np.asarray(r["out"], dtype=np.float64)
        cs = a.sum(axis=0)  # per-column partition sums
        for img in range(2):
            if img == 0:
                R = cs[0] + cs[1] + cs[2] + cs[3]
                row0, row1023 = a[0, 0], a[127, 3]
                E = cs[12:28].sum()
                e0 = a[0, 12] + a[0, 13]
                e1023 = a[127, 26] + a[127, 27]
            else:
                R = cs[4] + cs[5] + cs[6] + cs[7] + cs[8] + cs[9] \
                    + cs[10] + cs[11]
                row0, row1023 = a[0, 4], a[127, 9] + a[127, 10] + a[127, 11]
                E = cs[28:44].sum()
                e0 = a[0, 28] + a[0, 29]
                e1023 = a[127, 42] + a[127, 43]
            total += 3.0 * (3.0 * R - row0 - row1023) \
                - (3.0 * E - e0 - e1023)
    return np.float32(total)


def kernel(**inputs) -> np.ndarray:
    x = np.asarray(inputs["x"], dtype=np.float32)
    assert x.shape == (B, C, H, W), x.shape
    win = int(np.asarray(inputs.get("win_size", 3)))
    assert win == 3, f"kernel specialized for win_size=3, got {win}"

    if "nc" not in _CACHE:
        _CACHE["nc"] = build_nc()
    nc = _CACHE["nc"]

    res = run_bass_kernel_spmd(nc, make_in_maps(x), list(range(NCORES)))
    return np.array(_finish_host(res.results), dtype=np.float32)


# revision 11
# speedup vs baseline: 1.2162x; 1.0146x over previous
"""DCP pooling kernel for Trainium2 (8 NeuronCores, data-parallel over batch).

Math: reference pads x spatially, takes |min over channels| of the padded
image, sums all 3x3 sliding windows, then sums everything.  Padded zeros
contribute nothing, so the result collapses to

    sum_{b,h,w} |min_c x[b,c,h,w]| * rw(h) * cw(w)

with rw(h) = 2 if h in {0, H-1} else 3 (same for cw).  Pure streaming
reduction: read 192 MiB, emit one scalar -> memory-bound.

Layout: each 1024x1024 channel plane is viewed per half as [128, 4096]
(partition p holds 4 consecutive rows: image row = 512*h + 4*p + q,
flat col = q*1024 + w).  Every DMA descriptor is then 8KB of contiguous
DRAM per partition - measured ~408 GB/s per core vs ~344 GB/s for an
interleaved-channel 4KB-descriptor layout.

Device program per core (2 images = 4 half-planes = "chunks"):
  sync  (SP):   HWDGE loads, [128,2048] col-tiles x 3 channels per tile;
                the last half-plane is split into 6 shrinking col-pieces
                so the post-stream compute tail is tiny.  One shared load
                semaphore; consumers use cumulative per-load thresholds
                (min over channels starts when c0+c1 have landed).
  vector(DVE):  channel-min per tile (two tensor_tensor.min); edge-column
                extractions (|col 0| / |col 1023| per row-group) for
                chunks 0,1 and the pieces; last piece's rowsum.
  scalar(ACT):  Abs activation with fused accum_out per row-group slice
                -> per-(p,q)-rowsum staging cols; chunk 2's edge columns;
                issues the final [128,47] staging DMA to DRAM.
Host: applies the 2-vs-3 row/col weights in float64 from the staging
columns (rows 0/1023 get dedicated q-slices so their row sums are exact).
"""

import numpy as np

import concourse.bass as bass
import concourse.bacc as bacc
import concourse.mybir as mybir
from concourse.alu_op_type import AluOpType
from concourse.bass_utils import run_bass_kernel_spmd
from contextlib import ExitStack

B = 16            # full batch
NCORES = 8
BPC = B // NCORES  # images per core
C = 3
H = W = 1024
P = 128
F = 4096          # flat cols per half-plane: q*1024 + w, q = 0..3
f32 = mybir.dt.float32

# last half-plane (b=1, h=1) piece split (cols)
PIECES = [(0, 1024), (1024, 2048), (2048, 3072),
          (3072, 3584), (3584, 3840), (3840, 4096)]

# staging columns
# 0..13: ACT rowsum cols (per tile / piece, see build)
RSP5 = 14              # DVE rowsum of [3840:4096] of last half-plane
EDGE0 = 15             # chunk0 edges (DVE): q0c0,q0c1,q1c0,q1c1,q2c0,...
EDGE1 = 23             # chunk1 edges (DVE)
EDGE2 = 31             # chunk2 edges (ACT)
PEDGE = 39             # piece edges (DVE): q0c0,q0c1,q1c0,q1c1,q2c0,q2c1
Q3C0 = 45              # piece q3 col-0 edge (DVE)
Q3C1 = 46              # last piece's col-1023 edge (DVE)
NCOLS = 47

_CACHE: dict = {}


def build_nc() -> bass.Bass:
    nc = bacc.Bacc(detect_race_conditions=False)
    x = nc.declare_dram_parameter("x", [BPC, C, 2, P, F], f32, isOutput=False)
    out = nc.declare_dram_parameter("out", [P, NCOLS], f32, isOutput=True)

    # full tiles: (b, h, col range); chunk = 2*b + h; pieces cover (1,1)
    fulls = [(b, h, c0, c0 + 2048)
             for (b, h) in [(0, 0), (0, 1), (1, 0)] for c0 in (0, 2048)]

    with ExitStack() as ctx:
        ec = ctx.enter_context
        # 3 rotating trio slots for full tiles, [128, 3*2048] each
        slots = ec(nc.sbuf_tensor("slots", [P, 3 * 3 * 2048], f32))
        # pieces trio buffer for the (1,1) half-plane, [128, 3*4096]
        pslot = ec(nc.sbuf_tensor("pslot", [P, 3 * F], f32))
        t1 = ec(nc.sbuf_tensor("t1", [P, F], f32))
        m2a = ec(nc.sbuf_tensor("m2a", [P, F], f32))   # chunks 0, 2
        m2b = ec(nc.sbuf_tensor("m2b", [P, F], f32))   # chunk 1
        m2c = ec(nc.sbuf_tensor("m2c", [P, F], f32))   # chunk 3 (pieces)
        absout = ec(nc.sbuf_tensor("absout", [P, 2048], f32))
        stag = ec(nc.sbuf_tensor("stag", [P, NCOLS], f32))
        zbias = ec(nc.sbuf_tensor("zbias", [P, 1], f32))
        acksink = ec(nc.sbuf_tensor("acksink", [P, 1], f32))

        # Per-tile trio semaphores.  A wait threshold on a DMA semaphore is
        # only exact when it equals ALL increments ever issued on it (48 =
        # 16 SDMA engines x 3 channel loads) -- cumulative thresholds on a
        # shared semaphore raced (individual engines can lag a full load
        # behind the aggregate count).
        csem = [ec(nc.semaphore(f"csem{k}")) for k in range(12)]
        min2_done = ec(nc.semaphore("min2_done"))
        act_done = ec(nc.semaphore("act_done"))
        dve_fin = ec(nc.semaphore("dve_fin"))
        osem = ec(nc.semaphore("osem"))
        block = ec(nc.Block(no_gpsimd_drain=True))

        def slot_ap(k, cols):
            base = (k % 3) * 3 * 2048
            return [slots[:, base + c * 2048:base + c * 2048 + cols]
                    for c in range(C)]

        m2_of = [m2a, m2b, m2a]  # per chunk 0..2

        @block.sync
        def _(sync):
            for k, (b, h, c0, c1) in enumerate(fulls):
                if k >= 3:
                    # trio slot k%3 reused from tile k-3; freed by its min2
                    sync.wait_ge(min2_done, k - 2)
                dsts = slot_ap(k, c1 - c0)
                for c in range(C):
                    sync.dma_start(out=dsts[c], in_=x[b, c, h][:, c0:c1]
                                   ).then_inc(csem[k], 16)
            for j, (c0, c1) in enumerate(PIECES):
                for c in range(C):
                    sync.dma_start(out=pslot[:, c * F + c0:c * F + c1],
                                   in_=x[1, c, 1][:, c0:c1]
                                   ).then_inc(csem[6 + j], 16)

        @block.vector
        def _(vector):
            # zbias for ACT's Abs: ACT's first activation waits min2_done>=1,
            # which transitively orders it after this DVE memset.
            vector.memset(zbias[:], 0.0)

            def edge(col, m2, fc):
                vector.tensor_reduce(stag[:, col:col + 1], m2[:, fc:fc + 1],
                                     mybir.AxisListType.X, AluOpType.add,
                                     apply_absolute_value=True)

            for k, (b, h, c0, c1) in enumerate(fulls):
                chunk = 2 * b + h
                m2 = m2_of[chunk]
                s0, s1, s2 = slot_ap(k, c1 - c0)
                vector.wait_ge(csem[k], 48)
                vector.tensor_tensor(t1[:, c0:c1], s0, s1, AluOpType.min)
                if k == 4:
                    # m2a reuse: chunk-0 rowsum activations must have read it
                    vector.wait_ge(act_done, 3)
                vector.tensor_tensor(m2[:, c0:c1], t1[:, c0:c1], s2,
                                     AluOpType.min).then_inc(min2_done, 1)
                if chunk < 2:
                    # 2 q-groups per 2048-col tile -> 4 edge columns
                    base = EDGE0 if chunk == 0 else EDGE1
                    qbase = 0 if c0 == 0 else 2
                    for qq in range(2):
                        q = qbase + qq
                        edge(base + 2 * q, m2, q * 1024)
                        edge(base + 2 * q + 1, m2, q * 1024 + 1023)

            for j, (c0, c1) in enumerate(PIECES):
                vector.wait_ge(csem[6 + j], 48)
                vector.tensor_tensor(t1[:, c0:c1], pslot[:, c0:c1],
                                     pslot[:, F + c0:F + c1], AluOpType.min)
                vector.tensor_tensor(m2c[:, c0:c1], t1[:, c0:c1],
                                     pslot[:, 2 * F + c0:2 * F + c1],
                                     AluOpType.min).then_inc(min2_done, 1)
                if j < 3:
                    # pieces 0-2 are whole q-groups: both edge columns
                    edge(PEDGE + 2 * j, m2c, c0)
                    edge(PEDGE + 2 * j + 1, m2c, c0 + 1023)
                elif j == 3:
                    edge(Q3C0, m2c, 3072)
            # last piece epilogue: its rowsum + col-1023 edge
            vector.tensor_reduce(stag[:, RSP5:RSP5 + 1], m2c[:, 3840:4096],
                                 mybir.AxisListType.X, AluOpType.add,
                                 apply_absolute_value=True)
            vector.tensor_reduce(stag[:, Q3C1:Q3C1 + 1], m2c[:, 4095:4096],
                                 mybir.AxisListType.X, AluOpType.add,
                                 apply_absolute_value=True).then_inc(dve_fin, 1)

        @block.scalar
        def _(scalar):
            def act(col, m2, c0, c1, wait=None, inc=False):
                if wait is not None:
                    scalar.wait_ge(min2_done, wait)
                r = scalar.activation(absout[:, 0:c1 - c0], m2[:, c0:c1],
                                      mybir.ActivationFunctionType.Abs,
                                      bias=zbias[:],
                                      accum_out=stag[:, col:col + 1])
                if inc:
                    r.then_inc(act_done, 1)

            # bulk rowsums, gated per tile; h0 chunks isolate q0 (row 0),
            # h1 chunks isolate q3 (row 1023)
            act(0, m2a, 0, 1024, wait=1, inc=True)       # chunk0 q0
            act(1, m2a, 1024, 2048, inc=True)            # chunk0 q1
            act(2, m2a, 2048, 4096, wait=2, inc=True)    # chunk0 q23
            act(3, m2b, 0, 2048, wait=3)                 # chunk1 q01
            act(4, m2b, 2048, 3072, wait=4)              # chunk1 q2
            act(5, m2b, 3072, 4096)                      # chunk1 q3
            act(6, m2a, 0, 1024, wait=5)                 # chunk2 q0
            act(7, m2a, 1024, 2048)                      # chunk2 q1
            act(EDGE2 + 0, m2a, 0, 1)                    # chunk2 q0c0
            act(EDGE2 + 1, m2a, 1023, 1024)
            act(EDGE2 + 2, m2a, 1024, 1025)
            act(EDGE2 + 3, m2a, 2047, 2048)
            act(8, m2a, 2048, 4096, wait=6)              # chunk2 q23
            act(EDGE2 + 4, m2a, 2048, 2049)
            act(EDGE2 + 5, m2a, 3071, 3072)
            act(EDGE2 + 6, m2a, 3072, 3073)
            act(EDGE2 + 7, m2a, 4095, 4096)
            # piece rowsums
            act(9, m2c, 0, 1024, wait=7)
            act(10, m2c, 1024, 2048, wait=8)
            act(11, m2c, 2048, 3072, wait=9)
            act(12, m2c, 3072, 3584, wait=10)
            act(13, m2c, 3584, 3840, wait=11)
            # accum-retire guard: read the last accum col before trusting
            # any accum landed (walrus splits ACTIVATE/READ_ACCUMULATOR)
            scalar.copy(acksink[:], stag[:, 13:14])
            scalar.wait_ge(dve_fin, 1)
            scalar.dma_start(out=out[:], in_=stag[:]).then_inc(osem, 16)
            scalar.wait_ge(osem, 16)

    nc.finalize()
    return nc


def make_in_maps(x: np.ndarray) -> list:
    x = np.ascontiguousarray(np.asarray(x, dtype=np.float32))
    return [{"x": x[i * BPC:(i + 1) * BPC].reshape(BPC, C, 2, P, F)}
            for i in range(NCORES)]


def _finish_host(results) -> np.float32:
    total = 0.0
    for r in results:
        a = np.asarray(r["out"], dtype=np.float64)
        cs = a.sum(axis=0)  # per-column partition sums
        for img in range(2):
            if img == 0:
                R = cs[0:6].sum()
                row0, row1023 = a[0, 0], a[127, 5]
                E = cs[EDGE0:EDGE0 + 16].sum()
                e0 = a[0, EDGE0] + a[0, EDGE0 + 1]
                e1023 = a[127, EDGE1 + 6] + a[127, EDGE1 + 7]
            else:
                R = cs[6:15].sum()
                row0 = a[0, 6]
                row1023 = a[127, 12] + a[127, 13] + a[127, RSP5]
                E = cs[EDGE2:NCOLS].sum()
                e0 = a[0, EDGE2] + a[0, EDGE2 + 1]
                e1023 = a[127, Q3C0] + a[127, Q3C1]
            total += 3.0 * (3.0 * R - row0 - row1023) \
                - (3.0 * E - e0 - e1023)
    return np.float32(total)


def kernel(**inputs) -> np.ndarray:
    x = np.asarray(inputs["x"], dtype=np.float32)
    assert x.shape == (B, C, H, W), x.shape
    win = int(np.asarray(inputs.get("win_size", 3)))
    assert win == 3, f"kernel specialized for win_size=3, got {win}"

    if "nc" not in _CACHE:
        _CACHE["nc"] = build_nc()
    nc = _CACHE["nc"]

    res = run_bass_kernel_spmd(nc, make_in_maps(x), list(range(NCORES)))
    return np.array(_finish_host(res.results), dtype=np.float32)
